# revision 1
# baseline (speedup 1.0000x reference)
"""Trainium2 Bass kernel for nn_BalanceDropLoss (histogram_binning).

Math: for t in {0,1}, with s = t - 0.5 and v = s*x:
    bce  = softplus((1-2t)*x) = softplus(-2v) = ln(1 + exp(-2v))
    easy = |sigmoid(x)-t| < 1/BINS  <=>  v > ln(9)/2  <=>  u = exp(-2v) < 1/9
The loss only needs five per-class batch sums, computed in one pass:
    Ss  = sum(s)        T  = sum(bce)      Ssb = sum(s*bce)
    EB  = sum(e*bce)    Sse = sum(s*e*bce)          (e = easy indicator)
(the t-based sums follow as A = Ss + N/2, S1 = Ssb + T/2, TEB = Sse + EB/2).

Data-parallel over 8 cores (batch-sharded).  Elementwise products run on
the vector engine
in bf16 (2x mode); all per-class reductions run on the otherwise-idle
TensorEngine as ones-vector matmuls accumulating into PSUM (the matmul
window MMW = 8 rows x 40 classes is class-aligned, so every window of a
tensor accumulates into a single PSUM bank).  The tiny [5, C] combine
(class weights, majority/minority selection, final mean) runs on the host
in float64.
"""

import numpy as np

B_TOTAL = 524288
C = 40
NCORES = 8
P = 128
MMW = 320          # matmul window: 8 rows x 40 classes, fits one PSUM bank
NSUMS = 5
UEASY = 1.0 / 9.0  # exp(-ln 9): easy threshold in u-space
BAL = 0.5 * B_TOTAL


def _build(rows, rpp, repeats=1, bufs_in=3, bufs_mid=3):
    """Build the per-core SPMD program. rows = batch rows per core,
    rpp = rows per partition per tile (free width = rpp*C).
    repeats > 1 re-runs the whole pass (for slope-based HW timing)."""
    from contextlib import ExitStack


    import concourse.bass as bass  # noqa: F401  (registers engines)
    import concourse.tile as tile
    from concourse import bacc, mybir

    f32 = mybir.dt.float32
    bf16 = mybir.dt.bfloat16
    Act = mybir.ActivationFunctionType
    Alu = mybir.AluOpType

    F = rpp * C
    tile_rows = P * rpp
    ntiles = rows // tile_rows
    assert rows % tile_rows == 0 and F % MMW == 0
    nw = F // MMW

    nc = bacc.Bacc(
        "TRN2",
        target_bir_lowering=False,
        debug=False,
        num_devices=NCORES,
    )
    pred = nc.dram_tensor("pred", [rows, C], f32, kind="ExternalInput").ap()
    targ = nc.dram_tensor("target", [rows, C], f32, kind="ExternalInput").ap()
    out = nc.dram_tensor("out", [NSUMS, MMW], f32, kind="ExternalOutput").ap()

    pred_t = pred.rearrange("(n p f) c -> n p (f c)", p=P, f=rpp)
    targ_t = targ.rearrange("(n p f) c -> n p (f c)", p=P, f=rpp)

    with tile.TileContext(nc) as tc, ExitStack() as ctx:
        const_pool = ctx.enter_context(tc.tile_pool(name="const", bufs=1))
        in_pool = ctx.enter_context(tc.tile_pool(name="inp", bufs=bufs_in))
        mid_pool = ctx.enter_context(tc.tile_pool(name="mid", bufs=bufs_mid))
        psum_pool = ctx.enter_context(tc.tile_pool(name="acc", bufs=1, space="PSUM"))

        ones = const_pool.tile([P, 1], bf16)
        nc.vector.memset(ones[:], 1.0)

        # one [1, MMW] PSUM accumulator per reduced tensor (each 1 bank)
        accs = [
            psum_pool.tile([1, MMW], f32, name=f"acc{k}", tag=f"acc{k}")
            for k in range(NSUMS)
        ]

        for rep in range(repeats):
            for n in range(ntiles):
                xt = in_pool.tile([P, F], f32, tag="xt")
                nc.sync.dma_start(xt[:], pred_t[n])
                tt = in_pool.tile([P, F], f32, tag="tt")
                nc.sync.dma_start(tt[:], targ_t[n])

                x16 = mid_pool.tile([P, F], bf16, tag="x16")
                nc.vector.tensor_copy(x16[:], xt[:])
                s16 = mid_pool.tile([P, F], bf16, tag="s16")
                nc.scalar.activation(s16[:], tt[:], Act.Copy, bias=-0.5)
                v16 = mid_pool.tile([P, F], bf16, tag="v16")
                nc.vector.tensor_tensor(v16[:], s16[:], x16[:], op=Alu.mult)
                u16 = mid_pool.tile([P, F], bf16, tag="u16")
                nc.scalar.activation(u16[:], v16[:], Act.Exp, scale=-2.0)

                bce = mid_pool.tile([P, F], bf16, tag="bce")
                nc.scalar.activation(bce[:], u16[:], Act.Ln, bias=1.0)
                e16 = mid_pool.tile([P, F], bf16, tag="e16")
                nc.vector.tensor_scalar(e16[:], u16[:], UEASY, None, op0=Alu.is_lt)
                sb = mid_pool.tile([P, F], bf16, tag="sb")
                nc.vector.tensor_tensor(sb[:], s16[:], bce[:], op=Alu.mult)
                eb = mid_pool.tile([P, F], bf16, tag="eb")
                nc.vector.tensor_tensor(eb[:], e16[:], bce[:], op=Alu.mult)
                seb = mid_pool.tile([P, F], bf16, tag="seb")
                nc.vector.tensor_tensor(seb[:], s16[:], eb[:], op=Alu.mult)

                for k, tens in enumerate([s16, bce, sb, eb, seb]):
                    for w in range(nw):
                        nc.tensor.matmul(
                            accs[k][:, :],
                            ones[:, 0:1],
                            tens[:, w * MMW : (w + 1) * MMW],
                            start=(n == 0 and w == 0),
                            stop=(n == ntiles - 1 and w == nw - 1),
                            skip_group_check=repeats > 1,
                        )

        outsb = const_pool.tile([1, NSUMS * MMW], f32)
        for k in range(NSUMS):
            nc.scalar.copy(outsb[:, k * MMW : (k + 1) * MMW], accs[k][:, :])
        nc.sync.dma_start(out.rearrange("s m -> (s m)")[None, :], outsb[:])

    nc.compile()
    return nc


_NC_CACHE = {}


def _get_nc(rows, rpp):
    key = (rows, rpp)
    if key not in _NC_CACHE:
        _NC_CACHE[key] = _build(rows, rpp)
    return _NC_CACHE[key]


def _run(pred, target, rpp=64, trace=False, **kw):
    """Shard over cores, execute, return (per-core out arrays, raw results)."""
    from concourse.bass_utils import run_bass_kernel_spmd

    rows = pred.shape[0] // NCORES
    nc = _get_nc(rows, rpp)
    in_maps = [
        {
            "pred": np.ascontiguousarray(pred[i * rows : (i + 1) * rows]),
            "target": np.ascontiguousarray(target[i * rows : (i + 1) * rows]),
        }
        for i in range(NCORES)
    ]
    res = run_bass_kernel_spmd(nc, in_maps, list(range(NCORES)), trace=trace, **kw)
    outs = [res.results[i]["out"] for i in range(NCORES)]
    return outs, res


def _combine(outs, b_total=B_TOTAL):
    """Host-side: per-core [NSUMS, MMW] psum slots -> per-class sums -> loss."""
    S = np.zeros((NSUMS, C), dtype=np.float64)
    for o in outs:
        S += o.astype(np.float64).reshape(NSUMS, -1, C).sum(axis=1)
    Ss, T, Ssb, EB, Sse = S
    # de-shift the s = t - 0.5 sums
    A = Ss + b_total / 2.0
    S1 = Ssb + T / 2.0
    TEB = Sse + EB / 2.0
    bal = 0.5 * b_total
    neg = b_total - A
    pos_gt = A >= bal
    n_maj = np.where(pos_gt, A, neg)
    s_maj = np.where(pos_gt, S1, T - S1)
    g_maj = np.where(pos_gt, TEB, EB - TEB)
    n_min = np.where(pos_gt, neg, A)
    s_min = np.where(pos_gt, T - S1, S1)
    w_maj = bal / np.maximum(n_maj, 1.0)
    w_min = (b_total - bal) / np.maximum(n_min, 1.0)
    total = (w_maj * (s_maj - g_maj) + np.where(n_min > 0, w_min * s_min, 0.0)).sum()
    return np.float32(total / (b_total * C))


def kernel(pred: np.ndarray, target: np.ndarray) -> np.ndarray:
    pred = np.ascontiguousarray(pred, dtype=np.float32)
    target = np.ascontiguousarray(target, dtype=np.float32)
    outs, _ = _run(pred, target)
    return _combine(outs, b_total=pred.shape[0])

